# revision 1
# baseline (speedup 1.0000x reference)
"""Trainium2 Bass kernel for the NeRF-baby MLP (pointwise 7-layer MLP).

Data-parallel over 8 NeuronCores: each core processes N/8 points.

Per-core dataflow (per 1024-point chunk, feature-major working layout):
  - DMA x in natural layout [128 part, 8 pts x 6 ch] (batched, contiguous)
  - PE transpose [128, 48] -> PSUM [48, 128] (point-groups on rows)
  - layer 1 / layer-4-views as 4 "pair" matmuls (K=48 zero-padded lhsT)
  - layers 2,3,4b,5,6 as block-diagonal [128,128] matmuls over [128,512]
    rhs (two 512-point halves packed on partitions 0-63 / 64-127)
  - PSUM -> SBUF relu-copies split between ACT and DVE
  - layer 7 + sigma gather + point-major transpose fused into 8 small
    matmuls with the activations as the stationary operand
  - DMA out [128, 8 pts x 4 ch] (batched, contiguous)

Weights are tiny: packed host-side into zero-padded stationary tiles and
replicated to all cores. Matmul dtype selectable: bf16 (fast path),
f32r (fp32 bits, single-pass PE mode), or f32 (exact, 4x slower).
"""

import numpy as np
import ml_dtypes

import concourse.bass as bass
import concourse.bacc as bacc
import concourse.mybir as mybir
from concourse import tile
from concourse.bass_utils import run_bass_kernel_spmd
from concourse.vector_clock import ScopedClock

# ----------------------------------------------------------------------------
# Problem constants (hardcoded per harness contract)
# ----------------------------------------------------------------------------
N_TOTAL = 2097152
N_CORES = 8
PER_CORE = N_TOTAL // N_CORES  # 262144
HID = 64
CHUNK = 1024                    # points per compute chunk
XB = 32                         # chunks per input DMA batch
OB = 32                         # chunks per output DMA batch

DTYPE_MODE = "bf16"             # "bf16" | "f32r" | "f32"

AF = mybir.ActivationFunctionType


# ----------------------------------------------------------------------------
# Workaround: this walrus build accepts only <=2 sync waits on
# TPB_CTRL-class instructions (Drain/Nop). Tile's kernel-tail drain
# collects one wait per ticked semaphore and overflows. Spread the waits
# over a chain of nops, and cap waits on everything else too.
# ----------------------------------------------------------------------------
_MAX_CTRL_WAITS = 1
_MAX_OTHER_WAITS = 1
_PATCH_DONE = False


def _spread_waits(nc, inst, bb_insts, idx, max_keep):
    si = inst.sync_info
    if si is None or not si.on_wait or len(si.on_wait) <= max_keep:
        return 0
    waits = list(si.on_wait)
    si.on_wait = waits[:max_keep]
    rest = waits[max_keep:]
    ninserted = 0
    for i in range(0, len(rest), _MAX_CTRL_WAITS):
        chunk = rest[i : i + _MAX_CTRL_WAITS]
        nop = nc.engines[inst.engine].nop(hint="waitsplit", nofuse=True)
        cur = nc.cur_bb.bb.instructions
        assert cur[-1] is nop.ins
        cur.pop()
        import bass_rust as _br
        nop.ins.sync_info = _br.SyncInfo(on_wait=chunk, on_update=[])
        bb_insts.insert(idx + ninserted, nop.ins)
        ninserted += 1
    return ninserted


def _patched_drain_and_barrier(self, tick_clock, wait_clock):
    nc = self.nc
    drain_inst = nc.sync.drain()
    wait_clock.add_sem_waits(
        drain_inst.ins, ScopedClock({None: tick_clock.global_clock})
    )
    end_bb = nc.cur_bb.bb
    insts = end_bb.instructions
    assert insts[-1] is drain_inst.ins
    _spread_waits(nc, drain_inst.ins, insts, len(insts) - 1, _MAX_CTRL_WAITS)
    end_bb.instructions = insts

    nc.all_engine_barrier()
    assert self.sems is not None
    popped = nc._tile_sem_poison_stack.pop()
    assert popped is self._sem_poison
    nc.clear_and_free_semaphores(list(self.sems.allocated().values()))
    nc.all_engine_barrier()

    for f in nc.m.functions:
        for bb in f.blocks:
            bl = bb.instructions
            i = 0
            changed = False
            while i < len(bl):
                inst = bl[i]
                tname = type(inst).__name__
                cap = 1
                si = inst.sync_info
                if si is not None and si.on_wait and len(si.on_wait) > cap:
                    i += _spread_waits(nc, inst, bl, i, cap)
                    changed = True
                i += 1
            if changed:
                bb.instructions = bl


def _apply_patch():
    global _PATCH_DONE
    if not _PATCH_DONE:
        tile.TileContext._drain_and_barrier = _patched_drain_and_barrier
        _PATCH_DONE = True


# ----------------------------------------------------------------------------
# Host-side weight packing
# ----------------------------------------------------------------------------
def _np_dtype(mode):
    return ml_dtypes.bfloat16 if mode == "bf16" else np.float32


def _mm_dtype(mode):
    return {
        "bf16": mybir.dt.bfloat16,
        "f32r": mybir.dt.float32r,
        "f32": mybir.dt.float32,
    }[mode]


def pack_weights(pw0, pw1, pw2, cw0, cw1, cw2, cw3, mode):
    """Build zero-padded stationary operands. All fp32; cast at the end."""
    # interleave=4: partition p holds points 4p..4p+3; transpose gives
    # [24, 128] with point-class i at rows 6i..6i+5. Pair P packs class P
    # (cols 0-63) and class P+2 (cols 64-127).
    lw1 = np.zeros((2, 24, 128), np.float32)   # layer-1 pair matmuls
    lw4 = np.zeros((2, 24, 128), np.float32)   # layer-4 view-part pair matmuls
    for P in range(2):
        for half, blk in ((0, P), (1, P + 2)):
            r = 6 * blk
            c = 64 * half
            lw1[P, r : r + 3, c : c + 64] = pw0.T            # [3,64]
            lw4[P, r + 3 : r + 6, c : c + 64] = cw0[:, 0:3].T  # views -> c1
    # layer 3 has no relu: fold it into layer 4 (feat path) and the sigma
    # read-out. w4f = cw0_feat @ pw2_feat maps h2 -> c1 pre-activation.
    w4f = (cw0[:, 3:18] @ pw2[1:16, :]).astype(np.float32)    # [64, 64]
    bd = np.zeros((5, 128, 128), np.float32)
    for h in (0, 1):
        o = 64 * h
        bd[0, o : o + 64, o : o + 64] = pw1.T                 # layer 2
        bd[2, o : o + 64, o : o + 64] = w4f.T                 # folded 3+4 feat
        bd[3, o : o + 64, o : o + 64] = cw1.T                 # layer 5
        bd[4, o : o + 64, o : o + 64] = cw2.T                 # layer 6
    w7c = np.zeros((128, 8), np.float32)
    for h in (0, 1):
        w7c[64 * h : 64 * h + 64, 4 * h : 4 * h + 3] = cw3.T  # color
    w7s = np.zeros((128, 8), np.float32)
    w7s[0:64, 3] = pw2[0, :]                                  # sigma A from h2
    w7s[64:128, 7] = pw2[0, :]                                # sigma B from h2
    ident = np.eye(128, dtype=np.float32)
    npdt = _np_dtype(mode)
    return {
        "lw1": lw1.astype(npdt),
        "lw4": lw4.astype(npdt),
        "bd": bd.astype(npdt),
        "w7c": w7c.astype(npdt),
        "w7s": w7s.astype(npdt),
        "ident": ident.astype(npdt),
    }


# ----------------------------------------------------------------------------
# Bass kernel builder
# ----------------------------------------------------------------------------
def build_bass(per_core=PER_CORE, mode=DTYPE_MODE, xb=XB, ob=OB):
    assert per_core % CHUNK == 0
    nchunks = per_core // CHUNK
    xb = min(xb, nchunks)
    assert nchunks % xb == 0

    mmdt = _mm_dtype(mode)
    f32 = mybir.dt.float32

    nc = bacc.Bacc("TRN2", target_bir_lowering=False, debug=False)

    x_d = nc.dram_tensor("x", [per_core, 6], mmdt, kind="ExternalInput")
    y_d = nc.dram_tensor("y", [per_core, 4], f32, kind="ExternalOutput")
    lw1_d = nc.dram_tensor("lw1", [2, 24, 128], mmdt, kind="ExternalInput")
    lw4_d = nc.dram_tensor("lw4", [2, 24, 128], mmdt, kind="ExternalInput")
    bd_d = nc.dram_tensor("bd", [5, 128, 128], mmdt, kind="ExternalInput")
    w7c_d = nc.dram_tensor("w7c", [128, 8], mmdt, kind="ExternalInput")
    w7s_d = nc.dram_tensor("w7s", [128, 8], mmdt, kind="ExternalInput")
    id_d = nc.dram_tensor("ident", [128, 128], mmdt, kind="ExternalInput")

    # chunk = 1024 points = 2 subtiles (qq) of [128 partitions x 4 points]
    # x view: batch b -> [128, xb*48] with chunk t at cols 48t, subtile qq
    # at cols 48t + 24qq
    x_v = x_d.ap().rearrange(
        "(b t q p i) c -> b p t q (i c)", t=xb, q=2, p=128, i=4
    )
    # y view: point n = 1024t + 512qq + 4m + i -> partition m,
    # col 32t + 16qq + 4i + c
    y_v = y_d.ap().rearrange(
        "(b t q m i) c -> b m t q (i c)", t=xb, q=2, m=128, i=4
    )
    nbatch = nchunks // xb

    from contextlib import ExitStack

    with tile.TileContext(nc) as tc, ExitStack() as es:
        wpool = es.enter_context(tc.tile_pool(name="weights", bufs=1))
        lw1_sb = [wpool.tile([24, 128], mmdt, name=f"lw1_{t}", tag=f"lw1_{t}") for t in range(2)]
        lw4_sb = [wpool.tile([24, 128], mmdt, name=f"lw4_{t}", tag=f"lw4_{t}") for t in range(2)]
        bd_sb = [wpool.tile([128, 128], mmdt, name=f"bd_{i}", tag=f"bd_{i}") for i in range(5)]
        w7c_sb = wpool.tile([128, 8], mmdt, tag="w7c")
        w7s_sb = wpool.tile([128, 8], mmdt, tag="w7s")
        id_sb = wpool.tile([128, 128], mmdt, tag="ident")
        for t in range(2):
            nc.sync.dma_start(lw1_sb[t][:], lw1_d.ap()[t])
            nc.sync.dma_start(lw4_sb[t][:], lw4_d.ap()[t])
        for i in range(5):
            nc.sync.dma_start(bd_sb[i][:], bd_d.ap()[i])
        nc.sync.dma_start(w7c_sb[:], w7c_d.ap())
        nc.sync.dma_start(w7s_sb[:], w7s_d.ap())
        nc.sync.dma_start(id_sb[:], id_d.ap())

        xpool = es.enter_context(tc.tile_pool(name="xin", bufs=2))
        opool = es.enter_context(tc.tile_pool(name="oout", bufs=2))
        spool = es.enter_context(tc.tile_pool(name="work", bufs=2))
        pps = es.enter_context(tc.tile_pool(name="psl", bufs=1, space="PSUM"))
        ppx = es.enter_context(tc.tile_pool(name="psx", bufs=1, space="PSUM"))
        ppo = es.enter_context(tc.tile_pool(name="pso", bufs=2, space="PSUM"))

        def relu_copy(engine, dst, src):
            if engine == "act":
                nc.scalar.activation(dst, src, AF.Relu)
            else:
                nc.vector.tensor_scalar_max(dst, src, 0.0)

        def plain_copy(engine, dst, src):
            if engine == "act":
                nc.scalar.activation(dst, src, AF.Identity)
            else:
                nc.vector.tensor_copy(dst, src)

        # software-pipelined output stage: chunk t's 8 small matmuls are
        # emitted between chunk t+1's big matmuls so their LDWEIGHTS hide
        # under long streams. Each entry emits one (mm7a, mm7b) pair.
        pending = []

        def emit_pending(k):
            for _ in range(min(k, len(pending))):
                pending.pop(0)()

        def make_out_pair(u, c3_sb, h2_sb, out_r):
            def emit():
                o_ap = out_r[:, u & 1, u >> 1]
                nc.tensor.matmul(
                    o_ap, c3_sb[:, 128 * u : 128 * u + 128], w7c_sb[:],
                    start=True, stop=False, skip_group_check=True,
                )
                nc.tensor.matmul(
                    o_ap, h2_sb[:, 128 * u : 128 * u + 128], w7s_sb[:],
                    start=False, stop=True, skip_group_check=True,
                )
            return emit

        for b in range(nbatch):
            x_sb = xpool.tile([128, xb * 48], mmdt, tag="x")
            nc.sync.dma_start(x_sb[:], x_v[b])
            o_sb = opool.tile([128, xb * 32], f32, tag="o")
            for t in range(xb):
                # ---- input transpose: 2 subtiles of [128, 24] ----
                xt_ps = ppx.tile([24, 256], mmdt, tag="xt")
                for q in range(2):
                    nc.tensor.transpose(
                        xt_ps[:, 128 * q : 128 * q + 128],
                        x_sb[:, 48 * t + 24 * q : 48 * t + 24 * q + 24],
                        id_sb[:],
                    )
                xt_sb = spool.tile([24, 256], mmdt, tag="xt")
                plain_copy("dve", xt_sb[:], xt_ps[:])

                # ---- layer 1: 2 pair matmuls of N=256 ----
                h1_ps = pps.tile([128, 512], f32, tag="l_h1")
                for P in range(2):
                    nc.tensor.matmul(
                        h1_ps[:, 256 * P : 256 * P + 256],
                        lw1_sb[P][:], xt_sb[:],
                        start=True, stop=True,
                    )
                emit_pending(2)
                h1_sb = spool.tile([128, 512], mmdt, tag="h1")
                relu_copy("dve", h1_sb[:], h1_ps[:])

                # ---- layer 2 ----
                h2_ps = pps.tile([128, 512], f32, tag="l_h2")
                nc.tensor.matmul(h2_ps[:], bd_sb[0][:], h1_sb[:], start=True, stop=True)
                emit_pending(2)
                h2_sb = spool.tile([128, 512], mmdt, tag="h2")
                relu_copy("act", h2_sb[:], h2_ps[:])

                # ---- layer 4: views (2 pair mms) + folded 3+4 feat path ----
                c1_ps = pps.tile([128, 512], f32, tag="l_c1")
                for P in range(2):
                    nc.tensor.matmul(
                        c1_ps[:, 256 * P : 256 * P + 256],
                        lw4_sb[P][:], xt_sb[:],
                        start=(P == 0), stop=False, skip_group_check=True,
                    )
                nc.tensor.matmul(
                    c1_ps[:], bd_sb[2][:], h2_sb[:],
                    start=False, stop=True, skip_group_check=True,
                )
                emit_pending(2)
                c1_sb = spool.tile([128, 512], mmdt, tag="c1")
                relu_copy("act", c1_sb[:], c1_ps[:])

                # ---- layer 5 ----
                c2_ps = pps.tile([128, 512], f32, tag="l_c2")
                nc.tensor.matmul(c2_ps[:], bd_sb[3][:], c1_sb[:], start=True, stop=True)
                emit_pending(1)
                c2_sb = spool.tile([128, 512], mmdt, tag="c2")
                relu_copy("dve", c2_sb[:], c2_ps[:])

                # ---- layer 6 ----
                c3_ps = pps.tile([128, 512], f32, tag="l_c3")
                nc.tensor.matmul(c3_ps[:], bd_sb[4][:], c2_sb[:], start=True, stop=True)
                emit_pending(1)
                c3_sb = spool.tile([128, 512], mmdt, tag="c3")
                relu_copy("act", c3_sb[:], c3_ps[:])

                # ---- layer 7 + sigma, point-major, deferred ----
                out_ps = ppo.tile([128, 32], f32, tag="out")
                out_r = out_ps[:].rearrange("p (qq b2 a c) -> p qq a b2 c",
                                            qq=2, b2=2, a=2, c=4)
                for u in range(4):
                    pending.append(make_out_pair(u, c3_sb, h2_sb, out_r))
                osl = o_sb[:, 32 * t : 32 * t + 32]
                ops = out_ps

                def out_copy(osl=osl, ops=ops):
                    plain_copy("dve", osl, ops[:])
                pending.append(out_copy)
            emit_pending(len(pending))
            nc.sync.dma_start(y_v[b], o_sb[:])

    nc.compile()
    return nc


# ----------------------------------------------------------------------------
# Entry point
# ----------------------------------------------------------------------------
_CACHE = {}


def _get_nc(per_core, mode):
    key = (per_core, mode)
    if key not in _CACHE:
        _CACHE[key] = build_bass(per_core=per_core, mode=mode)
    return _CACHE[key]


def run(inputs, per_core=PER_CORE, mode=DTYPE_MODE, trace=False, **kw):
    """Shard inputs across 8 cores, run, gather. Returns (out, results)."""
    x = np.asarray(inputs["x"], np.float32)
    n = per_core * N_CORES
    w = pack_weights(
        np.asarray(inputs["pw0"], np.float32),
        np.asarray(inputs["pw1"], np.float32),
        np.asarray(inputs["pw2"], np.float32),
        np.asarray(inputs["cw0"], np.float32),
        np.asarray(inputs["cw1"], np.float32),
        np.asarray(inputs["cw2"], np.float32),
        np.asarray(inputs["cw3"], np.float32),
        mode,
    )
    xcast = x[:n].astype(_np_dtype(mode))
    in_maps = []
    for c in range(N_CORES):
        m = dict(w)
        m["x"] = np.ascontiguousarray(xcast[c * per_core : (c + 1) * per_core])
        in_maps.append(m)
    nc = _get_nc(per_core, mode)
    res = run_bass_kernel_spmd(nc, in_maps, list(range(N_CORES)), trace=trace, **kw)
    out = np.concatenate([res.results[c]["y"] for c in range(N_CORES)], axis=0)
    return out, res


def kernel(**inputs) -> np.ndarray:
    out, _ = run(inputs)
    return out

